# revision 21
# baseline (speedup 1.0000x reference)
"""CARAFE + MSGConv Trainium2 kernel (8 NeuronCores, spatial x batch sharding).

out[c, i, j] = sum_{p,q} W[5p+q, i, j] * Xpad[c, i//2 + p - 2, j//2 + q - 2]
 (CARAFE taps live at source resolution; identical for both subpixel parities).

Per core: one batch element (core//4) and a 16-source-row block (core%4).
The 25-tap reassembly runs on the TensorEngine as one K=120 matmul per
(row-pair, column-quarter) block:
  out[c, n] = sum_{(u,v)} X6T[(u,v), c] * B4[(u,v), n]
where B4 is a banded matrix of softmaxed W values built at runtime with
gpsimd local_scatter (per-partition index scatter) + a PE transpose; the
X side (X6T) is static data and comes pre-transposed from the host.
"""

import sys

sys.path.insert(0, "/opt/trn_rl_repo")

from contextlib import ExitStack

import ml_dtypes
import numpy as np

import concourse.bass as bass
import concourse.tile as tile
from concourse import bacc, library_config, mybir
from concourse.bass_utils import run_bass_kernel_spmd

BF16 = mybir.dt.bfloat16
F32 = mybir.dt.float32
I16 = mybir.dt.int16
AF = mybir.ActivationFunctionType
OP = mybir.AluOpType
nbf = ml_dtypes.bfloat16

C = 128
H = W = 64
NCORES = 8
XR = 24          # X shard rows (16 + 4 halo each side)
XW = 68          # padded width for dw slabs only
NEG = -30.0      # additive pre-activation mask; SiLU(-30) ~= -2.8e-12


# ======================================================================
# host-side parameter prep
# ======================================================================

def _fold_1x1(w, s):
    return (w[:, :, 0, 0] * s[:, None]).T.copy()


def _dw_taps(w, s, k):
    ch = w.shape[0]
    out = np.zeros((ch, 25), np.float32)
    off = (5 - k) // 2
    for ty in range(k):
        for tx in range(k):
            out[:, 5 * (ty + off) + (tx + off)] = w[:, 0, ty, tx] * s
    return out


def _host_consts(inputs):
    d = {}
    w_cv1 = _fold_1x1(inputs["comp_cv1_w"], inputs["comp_cv1_s"])
    b_cv1 = inputs["comp_cv1_b"].reshape(32, 1)
    w3 = _dw_taps(inputs["comp_dw3_w"], inputs["comp_dw3_s"], 3)
    w5 = _dw_taps(inputs["comp_dw5_w"], inputs["comp_dw5_s"], 5)
    w_dwp = np.tile(np.concatenate([w3, w5], 0), (4, 1))
    b_dwp = np.tile(
        np.concatenate([inputs["comp_dw3_b"], inputs["comp_dw5_b"]]), 4
    ).reshape(128, 1)
    w_px = _fold_1x1(inputs["comp_px_w"], inputs["comp_px_s"])
    b_px = inputs["comp_px_b"].reshape(64, 1)
    we = _fold_1x1(inputs["enc_cv1_w"], inputs["enc_cv1_s"])
    w_ecv1 = np.concatenate([we, np.ones((1, 50), np.float32)], 0)
    b_ecv1 = inputs["enc_cv1_b"].reshape(50, 1)
    e3 = _dw_taps(inputs["enc_dw3_w"], inputs["enc_dw3_s"], 3)
    e5 = _dw_taps(inputs["enc_dw5_w"], inputs["enc_dw5_s"], 5)
    w_edwp = np.tile(np.concatenate([e3, e5], 0), (2, 1))
    b_edwp = np.tile(
        np.concatenate([inputs["enc_dw3_b"], inputs["enc_dw5_b"]]), 2
    ).reshape(100, 1)
    wpx = _fold_1x1(inputs["enc_px_w"], inputs["enc_px_s"])
    w_epx = np.concatenate([wpx, inputs["enc_px_b"].reshape(1, 100)], 0)

    # packA bf16 [128, 374]: w_cv1 | w_px | w_ecv1 | w_epx | ident
    pa = np.zeros((128, 374), np.float32)
    pa[0:128, 0:32] = w_cv1
    pa[0:64, 32:96] = w_px
    pa[0:65, 96:146] = w_ecv1
    pa[0:101, 146:246] = w_epx
    pa[0:128, 246:374] = np.eye(128)
    d["packa"] = pa.astype(nbf)
    # packB f32 [128, 55]
    pb = np.zeros((128, 55), np.float32)
    pb[:, 0:25] = w_dwp
    pb[:, 25:26] = b_dwp
    pb[0:100, 26:51] = w_edwp
    pb[0:100, 51:52] = b_edwp
    pb[0:32, 52:53] = b_cv1
    pb[0:64, 53:54] = b_px
    pb[0:50, 54:55] = b_ecv1
    d["packb"] = pb

    d["ones1"] = np.ones((1, 32), nbf)
    d["erow1"] = np.ones((1, 16 * W), nbf)

    # repl [128, 4*128]: lhsT for the W row-replication matmul
    # n raster within a block: n = 32*(2*yl+dy) + (2*xl+dx)
    rp = np.zeros((128, 512), np.float32)
    for jb in range(4):
        for n in range(128):
            rho, j = divmod(n, 32)
            yl, xl = rho // 2, j // 2
            rp[64 * yl + 16 * jb + xl, 128 * jb + n] = 1.0
    d["repl"] = rp.astype(nbf)

    # sidx [128, 8*100] int16 (8 blocks per scatter call); horizontal
    # out-of-image taps are dropped here (-1 = skipped by local_scatter).
    si = np.full((128, 800), -1, np.int16)
    for n in range(128):
        rho, j = divmod(n, 32)
        yl, dy = divmod(rho, 2)
        xl, dx = divmod(j, 2)
        sn = 2 * dy + dx
        for bb in range(8):
            jb = bb % 4
            for cp in range(100):
                sc, k = divmod(cp, 25)
                if sc != sn:
                    continue
                p, q = divmod(k, 5)
                if not (0 <= 16 * jb + xl + q - 2 < 64):
                    continue
                si[n, 100 * bb + cp] = 120 * bb + 20 * (yl + p) + (xl + q)
    d["sidx"] = si
    return d


def _host_shard(X, core):
    b, ri = divmod(core, 4)
    r0 = 16 * ri - 4
    xs = np.zeros((C, XR, W), np.float32)
    lo, hi = max(0, r0), min(H, r0 + XR)
    xs[:, lo - r0 : hi - r0, :] = X[b, :, lo:hi, :]
    mrow = np.zeros((1, XR, W), np.float32)
    for r in range(XR):
        if not (0 <= r0 + r < H):
            mrow[0, r, :] = NEG
    emask = np.zeros((1, 20, W), np.float32)
    for r in range(20):
        if not (0 <= (16 * ri - 2) + r < H):
            emask[0, r, :] = NEG
    xsb = xs.astype(nbf)
    # pre-transposed X slabs, one [120, 128] per block (column-padded)
    xsp = np.zeros((C, XR, XW), nbf)
    xsp[:, :, 2 : 2 + W] = xsb
    xt = np.zeros((120, 32 * 128), nbf)
    for B in range(32):
        t, jb = divmod(B, 4)
        slab = xsp[:, 2 * t + 2 : 2 * t + 8, 16 * jb : 16 * jb + 20]
        xt[:, 128 * B : 128 * B + 128] = slab.reshape(C, 120).T
    return (
        xsb.reshape(C, XR * W),
        mrow.reshape(1, XR * W).astype(nbf),
        emask.reshape(1, 20 * W).astype(nbf),
        xt,
    )


# ======================================================================
# device kernel
# ======================================================================

def build_kernel():
    nc = bacc.Bacc(
        "TRN2",
        target_bir_lowering=False,
        debug=False,
        enable_asserts=False,
        num_devices=NCORES,
    )

    def din(name, shape, dt):
        return nc.dram_tensor(name, list(shape), dt, kind="ExternalInput").ap()

    x_d = din("x", (128, XR * W), BF16)
    xt_d = din("xt", (120, 32 * 128), BF16)
    mrow_d = din("mrow", (1, XR * W), BF16)
    emask_d = din("emask", (1, 20 * W), BF16)
    erow1_d = din("erow1", (1, 16 * W), BF16)
    ones1_d = din("ones1", (1, 32), BF16)
    packa_d = din("packa", (128, 374), BF16)
    packb_d = din("packb", (128, 55), F32)
    repl_d = din("repl", (128, 512), BF16)
    sidx_d = din("sidx", (128, 800), I16)
    out_d = nc.dram_tensor("out", [128, 32 * 128], F32, kind="ExternalOutput").ap()
    out3 = out_d.rearrange("c (r j) -> c r j", j=128)

    with tile.TileContext(nc) as tc, ExitStack() as ctx:
        cpool = ctx.enter_context(tc.tile_pool(name="consts", bufs=1))
        work = ctx.enter_context(tc.tile_pool(name="work", bufs=1))
        psA = ctx.enter_context(tc.tile_pool(name="psA", bufs=2, space="PSUM"))
        psB = ctx.enter_context(tc.tile_pool(name="psB", bufs=2, space="PSUM"))
        psO = ctx.enter_context(tc.tile_pool(name="psO", bufs=2, space="PSUM"))
        spool = ctx.enter_context(tc.tile_pool(name="stage", bufs=3))

        nc.gpsimd.load_library(library_config.local_scatter)

        def cload(ap_d, shape, dt, eng=None):
            t = cpool.tile(list(shape), dt, tag=ap_d.tensor.name)
            (eng or nc.sync).dma_start(t[:], ap_d)
            return t

        packa = cload(packa_d, (128, 374), BF16)
        packb = cload(packb_d, (128, 55), F32)
        mrow = cload(mrow_d, (1, XR * W), BF16, eng=nc.scalar)
        ones1 = cload(ones1_d, (1, 32), BF16, eng=nc.scalar)
        w_cv1 = packa[0:128, 0:32]
        w_px = packa[0:64, 32:96]
        w_ecv1 = packa[0:65, 96:146]
        w_epx = packa[0:101, 146:246]
        ident = packa[0:128, 246:374]
        w_dwp = packb[0:128, 0:25]
        b_dwp = packb[0:128, 25:26]
        w_edwp = packb[0:100, 26:51]
        b_edwp = packb[0:100, 51:52]
        b_cv1 = packb[0:32, 52:53]
        b_px = packb[0:64, 53:54]
        b_ecv1 = packb[0:50, 54:55]
        xb = cpool.tile([128, XR * W], BF16, tag="x")
        for ch in range(3):
            (nc.sync if ch != 1 else nc.scalar).dma_start(
                xb[:, 8 * W * ch : 8 * W * (ch + 1)],
                x_d[:, 8 * W * ch : 8 * W * (ch + 1)],
            )
        xt = cpool.tile([120, 32 * 128], BF16, tag="xt")
        for ch in range(2):
            nc.gpsimd.dma_start(
                xt[:, 2048 * ch : 2048 * (ch + 1)],
                xt_d[:, 2048 * ch : 2048 * (ch + 1)],
            )
        repl = cload(repl_d, (128, 512), BF16, eng=nc.gpsimd)
        sidx = cload(sidx_d, (128, 800), I16, eng=nc.gpsimd)

        # warmup: trigger the local_scatter ucode library load early so it
        # overlaps the conv front instead of stalling the first real scatter
        warm = work.tile([16, 16], BF16)
        nc.gpsimd.local_scatter(
            warm[:], packa[0:16, 0:2], sidx[:][0:16, 0:2],
            channels=16, num_elems=16, num_idxs=2,
        )

        # persistent working tensors (all 64-wide / contiguous)
        x12 = work.tile([64, XR * W], BF16)        # x1 (0:32) + x2 (32:64)
        enc_in = work.tile([65, 20 * W], BF16)     # px out + mask row
        enc_cat = work.tile([101, 16 * W], BF16)   # enc x1/x2 + ones row
        e1c = work.tile([50, 20 * W], BF16)        # enc cv1 out
        x1p = work.tile([128, 9 * XW + 8], BF16)   # packed x1 (68-pitch)
        e1p = work.tile([100, 12 * XW + 8], BF16)  # packed enc x1 (68-pitch)
        ET = work.tile([128, 800], F32)
        expv = work.tile([128, 800], F32)          # [s][t][k]
        S = work.tile([128, 32], F32)
        R = work.tile([128, 32], F32)
        wcat = work.tile([128, 800], BF16)         # [t][s][k]
        dall = work.tile([128, 3200], BF16)
        b4t = work.tile([128, 4 * 960], BF16)

        xb3 = xb[:].rearrange("p (r c) -> p r c", c=W)
        mrow3 = mrow[:].rearrange("p (r c) -> p r c", c=W)
        x12_3 = x12[:].rearrange("p (r c) -> p r c", c=W)
        enc_in3 = enc_in[:].rearrange("p (r c) -> p r c", c=W)
        enc_cat3 = enc_cat[:].rearrange("p (r c) -> p r c", c=W)
        e1c3 = e1c[:].rearrange("p (r c) -> p r c", c=W)
        x1p3 = x1p[:, 0 : 9 * XW].rearrange("p (r c) -> p r c", c=XW)
        e1p3 = e1p[:, 0 : 12 * XW].rearrange("p (r c) -> p r c", c=XW)
        ET3 = ET[:].rearrange("p (t e) -> p t e", e=100)
        exp3 = expv[:].rearrange("p (s t k) -> p s t k", s=4, t=8)

        # zero only the dw-slab pad columns (cols 0:2 and 66:68)
        nc.vector.memset(x1p[:, 9 * XW : 9 * XW + 8], 0.0)
        nc.vector.memset(e1p[:, 12 * XW : 12 * XW + 8], 0.0)
        nc.vector.memset(x1p3[:, :, 0:2], 0.0)
        nc.vector.memset(x1p3[:, :, 66:68], 0.0)
        nc.vector.memset(e1p3[:, :, 0:2], 0.0)
        nc.vector.memset(e1p3[:, :, 66:68], 0.0)
        nc.sync.dma_start(enc_cat[100:101, :], erow1_d)
        nc.sync.dma_start(enc_in[64:65, :], emask_d)

        # ---- comp cv1: 1x1 conv 128->32 (+ SiLU + out-of-image row mask)
        for ch in range(3):
            ps = psA.tile([32, 512], F32, tag="convps")
            nc.tensor.matmul(
                ps[:], w_cv1, xb[:, 512 * ch : 512 * (ch + 1)],
                start=True, stop=False,
            )
            nc.tensor.matmul(
                ps[:], ones1[:], mrow[:, 512 * ch : 512 * (ch + 1)],
                start=False, stop=True,
            )
            nc.scalar.activation(
                x12[0:32, 512 * ch : 512 * (ch + 1)], ps[:],
                AF.Silu, bias=b_cv1,
            )

        # ---- comp dw3/dw5 (unified 5x5 taps, rows packed 4x32)
        for g in range(4):
            nc.sync.dma_start(
                x1p3[32 * g : 32 * g + 32, 0:9, 2 : 2 + W],
                x12_3[0:32, 5 * g : 5 * g + 9, :],
            )
        FS = 5 * XW                    # 340
        acc_a = work.tile([128, FS], BF16)
        acc_b = work.tile([128, FS], BF16)
        accv = [acc_a[:], acc_b[:]]
        for t in range(25):
            ty, tx = divmod(t, 5)
            sv = x1p[:, ty * XW + tx : ty * XW + tx + FS]
            av = accv[t % 2]
            if t < 2:
                nc.vector.tensor_scalar(av, sv, w_dwp[:, t : t + 1], None, OP.mult)
            else:
                nc.vector.scalar_tensor_tensor(
                    av, sv, w_dwp[:, t : t + 1], av, OP.mult, OP.add
                )
        nc.vector.tensor_add(accv[0], accv[0], accv[1])
        x2p = work.tile([128, FS], BF16)
        nc.scalar.activation(x2p[:], acc_a[:], AF.Silu, bias=b_dwp)
        for g in range(4):
            nc.sync.dma_start(
                x12_3[32:64, 2 + 5 * g : 7 + 5 * g, :],
                x2p[32 * g : 32 * g + 32, :].rearrange(
                    "p (r c) -> p r c", c=XW
                )[:, 0:5, 0:W],
            )

        # ---- comp px: 1x1 conv 64->64 (+ SiLU)
        for r0, nr in ((0, 8), (8, 8), (16, 4)):
            ps = psA.tile([64, 512], F32, tag="convps")
            nc.tensor.matmul(
                ps[:, : nr * W], w_px,
                x12[0:64, (2 + r0) * W : (2 + r0 + nr) * W],
                start=True, stop=True,
            )
            nc.scalar.activation(
                enc_in[0:64, r0 * W : (r0 + nr) * W], ps[:, : nr * W],
                AF.Silu, bias=b_px,
            )

        # ---- enc cv1: 1x1 conv 64->50 (+ SiLU, mask row rides K=65)
        for r0, nr in ((0, 8), (8, 8), (16, 4)):
            ps = psA.tile([50, 512], F32, tag="convps")
            nc.tensor.matmul(
                ps[:, : nr * W], w_ecv1,
                enc_in[0:65, r0 * W : (r0 + nr) * W],
                start=True, stop=True,
            )
            nc.scalar.activation(
                e1c[0:50, r0 * W : (r0 + nr) * W], ps[:, : nr * W],
                AF.Silu, bias=b_ecv1,
            )

        # ---- enc dw3/dw5 (rows packed 2x50)
        for g in range(2):
            nc.sync.dma_start(
                e1p3[50 * g : 50 * g + 50, 0:12, 2 : 2 + W],
                e1c3[0:50, 8 * g : 8 * g + 12, :],
            )
        nc.sync.dma_start(enc_cat[0:50, :], e1c[0:50, 2 * W : 18 * W])
        FS2 = 8 * XW                   # 544
        acc2_a = work.tile([100, FS2], BF16)
        acc2_b = work.tile([100, FS2], BF16)
        acc2v = [acc2_a[:], acc2_b[:]]
        for t in range(25):
            ty, tx = divmod(t, 5)
            sv = e1p[:, ty * XW + tx : ty * XW + tx + FS2]
            av = acc2v[t % 2]
            if t < 2:
                nc.vector.tensor_scalar(av, sv, w_edwp[:, t : t + 1], None, OP.mult)
            else:
                nc.vector.scalar_tensor_tensor(
                    av, sv, w_edwp[:, t : t + 1], av, OP.mult, OP.add
                )
        nc.vector.tensor_add(acc2v[0], acc2v[0], acc2v[1])
        e2p = work.tile([100, FS2], BF16)
        nc.scalar.activation(e2p[:], acc2_a[:], AF.Silu, bias=b_edwp)
        for g in range(2):
            nc.sync.dma_start(
                enc_cat3[50:100, 8 * g : 8 * g + 8, :],
                e2p[50 * g : 50 * g + 50, :].rearrange(
                    "p (r c) -> p r c", c=XW
                )[:, 0:8, 0:W],
            )

        # ---- enc px (transposed output: M = 128 pixels per row-pair)
        for t in range(8):
            ps = psA.tile([128, 100], F32, tag="convps")
            nc.tensor.matmul(
                ps[:], enc_cat[0:101, 128 * t : 128 * t + 128],
                w_epx, start=True, stop=True,
            )
            nc.scalar.activation(ET[:, 100 * t : 100 * t + 100], ps[:], AF.Silu)

        # ---- softmax over 25 taps per subposition (no max-subtraction)
        for s in range(4):
            nc.scalar.activation(exp3[:, s], ET3[:, :, s::4], AF.Exp)
            nc.vector.tensor_reduce(
                S[:, 8 * s : 8 * s + 8], exp3[:, s], mybir.AxisListType.X, OP.add
            )
        nc.vector.reciprocal(R[:], S[:])
        for t in range(8):
            for s in range(4):
                nc.vector.tensor_scalar(
                    wcat[:, 100 * t + 25 * s : 100 * t + 25 * s + 25],
                    exp3[:, s, t],
                    R[:, 8 * s + t : 8 * s + t + 1],
                    None,
                    OP.mult,
                )

        # ---- block pipeline: replicate W rows -> scatter -> transpose -> MM
        for B in range(32):
            t, jb = divmod(B, 4)
            ps = psB.tile([128, 100], F32, tag="small")
            nc.tensor.matmul(
                ps[:], repl[:, 128 * jb : 128 * jb + 128],
                wcat[:, 100 * t : 100 * t + 100],
                start=True, stop=True,
            )
            dst = dall[:, 100 * B : 100 * B + 100]
            if B % 2 == 0:
                nc.vector.tensor_copy(dst, ps[:])
            else:
                nc.scalar.copy(dst, ps[:])

        for cl in range(4):
            nc.gpsimd.local_scatter(
                b4t[:, 960 * cl : 960 * cl + 960],
                dall[:, 800 * cl : 800 * cl + 800],
                sidx[:],
                channels=128, num_elems=960, num_idxs=800,
            )

        for B in range(32):
            t, jb = divmod(B, 4)
            cl, bb = divmod(B, 8)
            psb4 = psB.tile([120, 128], BF16, tag="b4t")
            nc.tensor.transpose(
                psb4[:], b4t[:, 960 * cl + 120 * bb : 960 * cl + 120 * bb + 120],
                ident,
            )
            b4 = spool.tile([120, 128], BF16, tag="b4")
            if B % 2 == 0:
                nc.scalar.copy(b4[:], psb4[:])
            else:
                nc.vector.tensor_copy(b4[:], psb4[:])

            po = psO.tile([128, 128], F32, tag="out")
            nc.tensor.matmul(
                po[:], xt[:, 128 * B : 128 * B + 128], b4[:],
                start=True, stop=True,
            )

            stg = spool.tile([128, 128], F32, tag="ostage")
            if B % 2 == 0:
                nc.vector.tensor_copy(stg[:], po[:])
            else:
                nc.scalar.copy(stg[:], po[:])
            oeng = (nc.sync, nc.scalar, nc.gpsimd)[B % 3]
            oeng.dma_start(
                out3[:, 4 * t : 4 * t + 4, 32 * jb : 32 * jb + 32],
                stg[:].rearrange("c (r j) -> c r j", j=32),
            )

    nc.compile()
    return nc


_NC_CACHE = None


def _get_nc():
    global _NC_CACHE
    if _NC_CACHE is None:
        _NC_CACHE = build_kernel()
    return _NC_CACHE


def kernel(**inputs) -> np.ndarray:
    X = np.asarray(inputs["X"], np.float32)
    consts = _host_consts(
        {k: np.asarray(v, np.float32) for k, v in inputs.items() if k != "X"}
    )
    in_maps = []
    for core in range(NCORES):
        xs, mrow, emask, xt = _host_shard(X, core)
        m = dict(consts)
        m["x"] = xs
        m["mrow"] = mrow
        m["emask"] = emask
        m["xt"] = xt
        in_maps.append(m)

    nc = _get_nc()
    res = run_bass_kernel_spmd(nc, in_maps, core_ids=list(range(NCORES)))
    out = np.zeros((2, C, 128, 128), np.float32)
    for core in range(NCORES):
        b, ri = divmod(core, 4)
        out[b, :, 32 * ri : 32 * ri + 32, :] = (
            res.results[core]["out"].reshape(C, 32, 128)
        )
    return out


if __name__ == "__main__":
    print("smoke build only")
    build_kernel()
    print("build ok")


# revision 25
# speedup vs baseline: 1.1847x; 1.1847x over previous
"""CARAFE + MSGConv Trainium2 kernel (8 NeuronCores, spatial x batch sharding).

out[c, i, j] = sum_{p,q} W[5p+q, i, j] * Xpad[c, i//2 + p - 2, j//2 + q - 2]
 (CARAFE taps live at source resolution; identical for both subpixel parities).

Per core: one batch element (core//4) and a 16-source-row block (core%4).
The 25-tap reassembly runs on the TensorEngine as one K=120 matmul per
(row-pair, column-quarter) block:
  out[c, n] = sum_{(u,v)} X6T[(u,v), c] * B4[(u,v), n]
where B4 is a banded matrix of softmaxed W values built at runtime with
gpsimd local_scatter (per-partition index scatter) + a PE transpose; the
X side (X6T) is static data and comes pre-transposed from the host.
"""

import sys

sys.path.insert(0, "/opt/trn_rl_repo")

from contextlib import ExitStack

import ml_dtypes
import numpy as np

import concourse.bass as bass
import concourse.tile as tile
from concourse import bacc, library_config, mybir
from concourse.bass_utils import run_bass_kernel_spmd

BF16 = mybir.dt.bfloat16
F32 = mybir.dt.float32
I16 = mybir.dt.int16
AF = mybir.ActivationFunctionType
OP = mybir.AluOpType
nbf = ml_dtypes.bfloat16

C = 128
H = W = 64
NCORES = 8
XR = 24          # X shard rows (16 + 4 halo each side)
XW = 68          # padded width for dw slabs only
NEG = -30.0      # additive pre-activation mask; SiLU(-30) ~= -2.8e-12


# ======================================================================
# host-side parameter prep
# ======================================================================

def _fold_1x1(w, s):
    return (w[:, :, 0, 0] * s[:, None]).T.copy()


def _dw_taps(w, s, k):
    ch = w.shape[0]
    out = np.zeros((ch, 25), np.float32)
    off = (5 - k) // 2
    for ty in range(k):
        for tx in range(k):
            out[:, 5 * (ty + off) + (tx + off)] = w[:, 0, ty, tx] * s
    return out


def _host_consts(inputs):
    d = {}
    w_cv1 = _fold_1x1(inputs["comp_cv1_w"], inputs["comp_cv1_s"])
    b_cv1 = inputs["comp_cv1_b"].reshape(32, 1)
    w3 = _dw_taps(inputs["comp_dw3_w"], inputs["comp_dw3_s"], 3)
    w5 = _dw_taps(inputs["comp_dw5_w"], inputs["comp_dw5_s"], 5)
    w_dwp = np.tile(np.concatenate([w3, w5], 0), (4, 1))
    b_dwp = np.tile(
        np.concatenate([inputs["comp_dw3_b"], inputs["comp_dw5_b"]]), 4
    ).reshape(128, 1)
    w_px = _fold_1x1(inputs["comp_px_w"], inputs["comp_px_s"])
    b_px = inputs["comp_px_b"].reshape(64, 1)
    we = _fold_1x1(inputs["enc_cv1_w"], inputs["enc_cv1_s"])
    w_ecv1 = np.concatenate([we, np.ones((1, 50), np.float32)], 0)
    b_ecv1 = inputs["enc_cv1_b"].reshape(50, 1)
    e3 = _dw_taps(inputs["enc_dw3_w"], inputs["enc_dw3_s"], 3)
    e5 = _dw_taps(inputs["enc_dw5_w"], inputs["enc_dw5_s"], 5)
    w_edwp = np.tile(np.concatenate([e3, e5], 0), (2, 1))
    b_edwp = np.tile(
        np.concatenate([inputs["enc_dw3_b"], inputs["enc_dw5_b"]]), 2
    ).reshape(100, 1)
    wpx = _fold_1x1(inputs["enc_px_w"], inputs["enc_px_s"])
    w_epx = np.concatenate([wpx, inputs["enc_px_b"].reshape(1, 100)], 0)

    # packA bf16 [128, 374]: w_cv1 | w_px | w_ecv1 | w_epx | ident
    pa = np.zeros((128, 374), np.float32)
    pa[0:128, 0:32] = w_cv1
    pa[0:64, 32:96] = w_px
    pa[0:65, 96:146] = w_ecv1
    pa[0:101, 146:246] = w_epx
    pa[0:128, 246:374] = np.eye(128)
    d["packa"] = pa.astype(nbf)
    # packB f32 [128, 55]
    pb = np.zeros((128, 55), np.float32)
    pb[:, 0:25] = w_dwp
    pb[:, 25:26] = b_dwp
    pb[0:100, 26:51] = w_edwp
    pb[0:100, 51:52] = b_edwp
    pb[0:32, 52:53] = b_cv1
    pb[0:64, 53:54] = b_px
    pb[0:50, 54:55] = b_ecv1
    d["packb"] = pb

    d["ones1"] = np.ones((1, 32), nbf)
    d["erow1"] = np.ones((1, 16 * W), nbf)

    # repl [128, 4*128]: lhsT for the W row-replication matmul
    # n raster within a block: n = 32*(2*yl+dy) + (2*xl+dx)
    rp = np.zeros((128, 512), np.float32)
    for jb in range(4):
        for n in range(128):
            rho, j = divmod(n, 32)
            yl, xl = rho // 2, j // 2
            rp[64 * yl + 16 * jb + xl, 128 * jb + n] = 1.0
    d["repl"] = rp.astype(nbf)

    # sidx [128, 8*100] int16 (8 blocks per scatter call); horizontal
    # out-of-image taps are dropped here (-1 = skipped by local_scatter).
    si = np.full((128, 800), -1, np.int16)
    for n in range(128):
        rho, j = divmod(n, 32)
        yl, dy = divmod(rho, 2)
        xl, dx = divmod(j, 2)
        sn = 2 * dy + dx
        for bb in range(8):
            jb = bb % 4
            for cp in range(100):
                sc, k = divmod(cp, 25)
                if sc != sn:
                    continue
                p, q = divmod(k, 5)
                if not (0 <= 16 * jb + xl + q - 2 < 64):
                    continue
                si[n, 100 * bb + cp] = 120 * bb + 20 * (yl + p) + (xl + q)
    d["sidx"] = si
    return d


def _host_shard(X, core):
    b, ri = divmod(core, 4)
    r0 = 16 * ri - 4
    xs = np.zeros((C, XR, W), np.float32)
    lo, hi = max(0, r0), min(H, r0 + XR)
    xs[:, lo - r0 : hi - r0, :] = X[b, :, lo:hi, :]
    mrow = np.zeros((1, XR, W), np.float32)
    for r in range(XR):
        if not (0 <= r0 + r < H):
            mrow[0, r, :] = NEG
    emask = np.zeros((1, 20, W), np.float32)
    for r in range(20):
        if not (0 <= (16 * ri - 2) + r < H):
            emask[0, r, :] = NEG
    xsb = xs.astype(nbf)
    # pre-transposed X slabs, one [120, 128] per block (column-padded)
    xsp = np.zeros((C, XR, XW), nbf)
    xsp[:, :, 2 : 2 + W] = xsb
    xt = np.zeros((120, 32 * 128), nbf)
    for B in range(32):
        t, jb = divmod(B, 4)
        slab = xsp[:, 2 * t + 2 : 2 * t + 8, 16 * jb : 16 * jb + 20]
        xt[:, 128 * B : 128 * B + 128] = slab.reshape(C, 120).T
    return (
        xsb.reshape(C, XR * W),
        mrow.reshape(1, XR * W).astype(nbf),
        emask.reshape(1, 20 * W).astype(nbf),
        xt,
    )


# ======================================================================
# device kernel
# ======================================================================

def build_kernel():
    nc = bacc.Bacc(
        "TRN2",
        target_bir_lowering=False,
        debug=False,
        enable_asserts=False,
        num_devices=NCORES,
    )

    def din(name, shape, dt):
        return nc.dram_tensor(name, list(shape), dt, kind="ExternalInput").ap()

    x_d = din("x", (128, XR * W), BF16)
    xt_d = din("xt", (120, 32 * 128), BF16)
    mrow_d = din("mrow", (1, XR * W), BF16)
    emask_d = din("emask", (1, 20 * W), BF16)
    erow1_d = din("erow1", (1, 16 * W), BF16)
    ones1_d = din("ones1", (1, 32), BF16)
    packa_d = din("packa", (128, 374), BF16)
    packb_d = din("packb", (128, 55), F32)
    repl_d = din("repl", (128, 512), BF16)
    sidx_d = din("sidx", (128, 800), I16)
    out_d = nc.dram_tensor("out", [128, 32 * 128], F32, kind="ExternalOutput").ap()
    out3 = out_d.rearrange("c (r j) -> c r j", j=128)

    with tile.TileContext(nc) as tc, ExitStack() as ctx:
        cpool = ctx.enter_context(tc.tile_pool(name="consts", bufs=1))
        work = ctx.enter_context(tc.tile_pool(name="work", bufs=1))
        psB = ctx.enter_context(tc.tile_pool(name="psB", bufs=2, space="PSUM"))
        spool = ctx.enter_context(tc.tile_pool(name="stage", bufs=3))
        psA_cm = tc.tile_pool(name="psA", bufs=2, space="PSUM")
        psA = psA_cm.__enter__()

        nc.gpsimd.load_library(library_config.local_scatter)

        def cload(ap_d, shape, dt, eng=None):
            t = cpool.tile(list(shape), dt, tag=ap_d.tensor.name)
            (eng or nc.sync).dma_start(t[:], ap_d)
            return t

        packa = cload(packa_d, (128, 374), BF16)
        packb = cload(packb_d, (128, 55), F32)
        mrow = cload(mrow_d, (1, XR * W), BF16, eng=nc.scalar)
        ones1 = cload(ones1_d, (1, 32), BF16, eng=nc.scalar)
        w_cv1 = packa[0:128, 0:32]
        w_px = packa[0:64, 32:96]
        w_ecv1 = packa[0:65, 96:146]
        w_epx = packa[0:101, 146:246]
        ident = packa[0:128, 246:374]
        w_dwp = packb[0:128, 0:25]
        b_dwp = packb[0:128, 25:26]
        w_edwp = packb[0:100, 26:51]
        b_edwp = packb[0:100, 51:52]
        b_cv1 = packb[0:32, 52:53]
        b_px = packb[0:64, 53:54]
        b_ecv1 = packb[0:50, 54:55]
        xb = cpool.tile([128, XR * W], BF16, tag="x")
        for ch in range(3):
            (nc.sync if ch != 1 else nc.scalar).dma_start(
                xb[:, 8 * W * ch : 8 * W * (ch + 1)],
                x_d[:, 8 * W * ch : 8 * W * (ch + 1)],
            )
        xt = cpool.tile([120, 32 * 128], BF16, tag="xt")
        for ch in range(2):
            nc.gpsimd.dma_start(
                xt[:, 2048 * ch : 2048 * (ch + 1)],
                xt_d[:, 2048 * ch : 2048 * (ch + 1)],
            )
        repl = cload(repl_d, (128, 512), BF16, eng=nc.gpsimd)
        sidx = cload(sidx_d, (128, 800), I16, eng=nc.gpsimd)

        # warmup: trigger the local_scatter ucode library load early so it
        # overlaps the conv front instead of stalling the first real scatter
        warm = work.tile([16, 16], BF16)
        nc.gpsimd.local_scatter(
            warm[:], packa[0:16, 0:2], sidx[:][0:16, 0:2],
            channels=16, num_elems=16, num_idxs=2,
        )

        # persistent working tensors (all 64-wide / contiguous)
        x12 = work.tile([64, XR * W], BF16)        # x1 (0:32) + x2 (32:64)
        enc_in = work.tile([65, 20 * W], BF16)     # px out + mask row
        enc_cat = work.tile([101, 16 * W], BF16)   # enc x1/x2 + ones row
        e1c = work.tile([50, 20 * W], BF16)        # enc cv1 out
        x1p = work.tile([128, 9 * XW + 8], BF16)   # packed x1 (68-pitch)
        e1p = work.tile([100, 12 * XW + 8], BF16)  # packed enc x1 (68-pitch)
        ET = work.tile([128, 800], F32)
        expv = work.tile([128, 800], F32)          # [s][t][k]
        S = work.tile([128, 32], F32)
        R = work.tile([128, 32], F32)
        wcat = work.tile([128, 800], BF16)         # [t][s][k]
        dall = work.tile([128, 3200], BF16)
        b4t = work.tile([128, 4 * 960], BF16)

        xb3 = xb[:].rearrange("p (r c) -> p r c", c=W)
        mrow3 = mrow[:].rearrange("p (r c) -> p r c", c=W)
        x12_3 = x12[:].rearrange("p (r c) -> p r c", c=W)
        enc_in3 = enc_in[:].rearrange("p (r c) -> p r c", c=W)
        enc_cat3 = enc_cat[:].rearrange("p (r c) -> p r c", c=W)
        e1c3 = e1c[:].rearrange("p (r c) -> p r c", c=W)
        x1p3 = x1p[:, 0 : 9 * XW].rearrange("p (r c) -> p r c", c=XW)
        e1p3 = e1p[:, 0 : 12 * XW].rearrange("p (r c) -> p r c", c=XW)
        ET3 = ET[:].rearrange("p (t e) -> p t e", e=100)
        exp3 = expv[:].rearrange("p (s t k) -> p s t k", s=4, t=8)

        # zero only the dw-slab pad columns (cols 0:2 and 66:68)
        nc.vector.memset(x1p[:, 9 * XW : 9 * XW + 8], 0.0)
        nc.vector.memset(e1p[:, 12 * XW : 12 * XW + 8], 0.0)
        nc.vector.memset(x1p3[:, :, 0:2], 0.0)
        nc.vector.memset(x1p3[:, :, 66:68], 0.0)
        nc.vector.memset(e1p3[:, :, 0:2], 0.0)
        nc.vector.memset(e1p3[:, :, 66:68], 0.0)
        nc.sync.dma_start(enc_cat[100:101, :], erow1_d)
        nc.sync.dma_start(enc_in[64:65, :], emask_d)

        # ---- comp cv1: 1x1 conv 128->32 (+ SiLU + out-of-image row mask)
        for ch in range(3):
            ps = psA.tile([32, 512], F32, tag="convps")
            nc.tensor.matmul(
                ps[:], w_cv1, xb[:, 512 * ch : 512 * (ch + 1)],
                start=True, stop=False,
            )
            nc.tensor.matmul(
                ps[:], ones1[:], mrow[:, 512 * ch : 512 * (ch + 1)],
                start=False, stop=True,
            )
            nc.scalar.activation(
                x12[0:32, 512 * ch : 512 * (ch + 1)], ps[:],
                AF.Silu, bias=b_cv1,
            )

        # ---- comp dw3/dw5 (unified 5x5 taps, rows packed 4x32)
        for g in range(4):
            nc.sync.dma_start(
                x1p3[32 * g : 32 * g + 32, 0:9, 2 : 2 + W],
                x12_3[0:32, 5 * g : 5 * g + 9, :],
            )
        FS = 5 * XW                    # 340
        acc_a = work.tile([128, FS], BF16)
        acc_b = work.tile([128, FS], BF16)
        accv = [acc_a[:], acc_b[:]]
        for t in range(25):
            ty, tx = divmod(t, 5)
            sv = x1p[:, ty * XW + tx : ty * XW + tx + FS]
            av = accv[t % 2]
            if t < 2:
                nc.vector.tensor_scalar(av, sv, w_dwp[:, t : t + 1], None, OP.mult)
            else:
                nc.vector.scalar_tensor_tensor(
                    av, sv, w_dwp[:, t : t + 1], av, OP.mult, OP.add
                )
        nc.vector.tensor_add(accv[0], accv[0], accv[1])
        x2p = work.tile([128, FS], BF16)
        nc.scalar.activation(x2p[:], acc_a[:], AF.Silu, bias=b_dwp)
        for g in range(4):
            nc.sync.dma_start(
                x12_3[32:64, 2 + 5 * g : 7 + 5 * g, :],
                x2p[32 * g : 32 * g + 32, :].rearrange(
                    "p (r c) -> p r c", c=XW
                )[:, 0:5, 0:W],
            )

        # ---- comp px: 1x1 conv 64->64 (+ SiLU)
        for r0, nr in ((0, 8), (8, 8), (16, 4)):
            ps = psA.tile([64, 512], F32, tag="convps")
            nc.tensor.matmul(
                ps[:, : nr * W], w_px,
                x12[0:64, (2 + r0) * W : (2 + r0 + nr) * W],
                start=True, stop=True,
            )
            nc.scalar.activation(
                enc_in[0:64, r0 * W : (r0 + nr) * W], ps[:, : nr * W],
                AF.Silu, bias=b_px,
            )

        # ---- enc cv1: 1x1 conv 64->50 (+ SiLU, mask row rides K=65)
        for r0, nr in ((0, 8), (8, 8), (16, 4)):
            ps = psA.tile([50, 512], F32, tag="convps")
            nc.tensor.matmul(
                ps[:, : nr * W], w_ecv1,
                enc_in[0:65, r0 * W : (r0 + nr) * W],
                start=True, stop=True,
            )
            nc.scalar.activation(
                e1c[0:50, r0 * W : (r0 + nr) * W], ps[:, : nr * W],
                AF.Silu, bias=b_ecv1,
            )

        # ---- enc dw3/dw5 (rows packed 2x50)
        for g in range(2):
            nc.sync.dma_start(
                e1p3[50 * g : 50 * g + 50, 0:12, 2 : 2 + W],
                e1c3[0:50, 8 * g : 8 * g + 12, :],
            )
        nc.sync.dma_start(enc_cat[0:50, :], e1c[0:50, 2 * W : 18 * W])
        FS2 = 8 * XW                   # 544
        acc2_a = work.tile([100, FS2], BF16)
        acc2_b = work.tile([100, FS2], BF16)
        acc2v = [acc2_a[:], acc2_b[:]]
        for t in range(25):
            ty, tx = divmod(t, 5)
            sv = e1p[:, ty * XW + tx : ty * XW + tx + FS2]
            av = acc2v[t % 2]
            if t < 2:
                nc.vector.tensor_scalar(av, sv, w_edwp[:, t : t + 1], None, OP.mult)
            else:
                nc.vector.scalar_tensor_tensor(
                    av, sv, w_edwp[:, t : t + 1], av, OP.mult, OP.add
                )
        nc.vector.tensor_add(acc2v[0], acc2v[0], acc2v[1])
        e2p = work.tile([100, FS2], BF16)
        nc.scalar.activation(e2p[:], acc2_a[:], AF.Silu, bias=b_edwp)
        for g in range(2):
            nc.sync.dma_start(
                enc_cat3[50:100, 8 * g : 8 * g + 8, :],
                e2p[50 * g : 50 * g + 50, :].rearrange(
                    "p (r c) -> p r c", c=XW
                )[:, 0:8, 0:W],
            )

        # ---- enc px (transposed output: M = 128 pixels per row-pair)
        for t in range(8):
            ps = psA.tile([128, 100], F32, tag="convps")
            nc.tensor.matmul(
                ps[:], enc_cat[0:101, 128 * t : 128 * t + 128],
                w_epx, start=True, stop=True,
            )
            nc.scalar.activation(ET[:, 100 * t : 100 * t + 100], ps[:], AF.Silu)

        # ---- softmax over 25 taps per subposition (no max-subtraction)
        for s in range(4):
            nc.scalar.activation(exp3[:, s], ET3[:, :, s::4], AF.Exp)
            nc.vector.tensor_reduce(
                S[:, 8 * s : 8 * s + 8], exp3[:, s], mybir.AxisListType.X, OP.add
            )
        nc.vector.reciprocal(R[:], S[:])
        psA_cm.__exit__(None, None, None)
        psO = ctx.enter_context(tc.tile_pool(name="psO", bufs=3, space="PSUM"))
        for t in range(8):
            for s in range(4):
                dstw = wcat[:, 100 * t + 25 * s : 100 * t + 25 * s + 25]
                if (4 * t + s) % 2 == 0:
                    nc.vector.tensor_scalar(
                        dstw, exp3[:, s, t],
                        R[:, 8 * s + t : 8 * s + t + 1], None, OP.mult,
                    )
                else:
                    nc.scalar.activation(
                        dstw, exp3[:, s, t], AF.Copy,
                        scale=R[:, 8 * s + t : 8 * s + t + 1],
                    )

        # ---- block pipeline: replicate W rows -> scatter -> transpose -> MM
        for B in range(32):
            t, jb = divmod(B, 4)
            ps = psB.tile([128, 100], F32, tag="small")
            nc.tensor.matmul(
                ps[:], repl[:, 128 * jb : 128 * jb + 128],
                wcat[:, 100 * t : 100 * t + 100],
                start=True, stop=True,
            )
            dst = dall[:, 100 * B : 100 * B + 100]
            if B % 2 == 0:
                nc.vector.tensor_copy(dst, ps[:])
            else:
                nc.scalar.copy(dst, ps[:])

        for cl in range(4):
            nc.gpsimd.local_scatter(
                b4t[:, 960 * cl : 960 * cl + 960],
                dall[:, 800 * cl : 800 * cl + 800],
                sidx[:],
                channels=128, num_elems=960, num_idxs=800,
            )

        stgs = []
        for B in range(32):
            t, jb = divmod(B, 4)
            cl, bb = divmod(B, 8)
            psb4 = psB.tile([120, 128], BF16, tag="b4t")
            nc.tensor.transpose(
                psb4[:], b4t[:, 960 * cl + 120 * bb : 960 * cl + 120 * bb + 120],
                ident,
            )
            b4 = spool.tile([120, 128], BF16, tag="b4")
            if B % 2 == 0:
                nc.scalar.copy(b4[:], psb4[:])
            else:
                nc.vector.tensor_copy(b4[:], psb4[:])

            po = psO.tile([128, 128], F32, tag="out")
            nc.tensor.matmul(
                po[:], xt[:, 128 * B : 128 * B + 128], b4[:],
                start=True, stop=True,
            )

            if jb == 0:
                stg = spool.tile([128, 512], F32, tag="ostage")
                stgs.append(stg)
            stg = stgs[-1]
            sv_dst = stg[:].rearrange("c (r j) -> c r j", j=128)[
                :, :, 32 * jb : 32 * jb + 32
            ]
            if B % 2 == 0:
                nc.vector.tensor_copy(
                    sv_dst, po[:].rearrange("c (r j) -> c r j", j=32)
                )
            else:
                nc.scalar.copy(
                    sv_dst, po[:].rearrange("c (r j) -> c r j", j=32)
                )
            if jb == 3:
                (nc.sync if t % 2 == 0 else nc.scalar).dma_start(
                    out3[:, 4 * t : 4 * t + 4, :],
                    stg[:].rearrange("c (r j) -> c r j", j=128),
                )

    nc.compile()
    return nc


_NC_CACHE = None


def _get_nc():
    global _NC_CACHE
    if _NC_CACHE is None:
        _NC_CACHE = build_kernel()
    return _NC_CACHE


def kernel(**inputs) -> np.ndarray:
    X = np.asarray(inputs["X"], np.float32)
    consts = _host_consts(
        {k: np.asarray(v, np.float32) for k, v in inputs.items() if k != "X"}
    )
    in_maps = []
    for core in range(NCORES):
        xs, mrow, emask, xt = _host_shard(X, core)
        m = dict(consts)
        m["x"] = xs
        m["mrow"] = mrow
        m["emask"] = emask
        m["xt"] = xt
        in_maps.append(m)

    nc = _get_nc()
    res = run_bass_kernel_spmd(nc, in_maps, core_ids=list(range(NCORES)))
    out = np.zeros((2, C, 128, 128), np.float32)
    for core in range(NCORES):
        b, ri = divmod(core, 4)
        out[b, :, 32 * ri : 32 * ri + 32, :] = (
            res.results[core]["out"].reshape(C, 32, 128)
        )
    return out


if __name__ == "__main__":
    print("smoke build only")
    build_kernel()
    print("build ok")
